# revision 15
# baseline (speedup 1.0000x reference)
"""Causal GQA self-attention (RMS-normed QK + RoPE + softmax + proj) on 8 trn2 cores.

Sharding: core c = (batch b = c//2, head-group g = c%2).  Each core computes
batch b, q-heads {8g..8g+7}, kv-heads {2g, 2g+1}, and a partial output
projection using Wproj columns for those heads; the host sums the two
partials per batch.

Device layout notes (per core):
 - All matmul operands bf16; accumulation fp32 in PSUM.
 - x is pre-transposed on host: xT [D=1024, S=2048].
 - Q/K are projected into [s, dim] layout (for free-dim RMS-norm + RoPE),
   then PE-transposed into [dim, s] for attention.
 - Local q-dim order pairs head m of group0 with head m of group1:
   [h0,h4,h1,h5,h2,h6,h3,h7] so scores for the two kv-heads can be computed
   with two row-tiled (K=64) matmuls sharing the PE array.
 - Scores are computed TRANSPOSED: S^T [k_s, (pair m, q)] so exp output
   feeds PV directly as the moving operand; V is augmented with a ones
   column so row 64 of the PV accumulator is the softmax denominator.
 - Single fused loop: prep of chunk i+1 (QKV proj / norm / rope / transpose)
   is interleaved with the attention row i and the deferred projection of
   chunk i-1, sharing one rotating PSUM pool so the PE array never drains
   (keeps the HAM clock gate at 2.4 GHz).
 - rsqrt for RMS norm = exp(-0.5*ln(v)); ln and exp share one ACT table set
   (natural_log_exp_and_others) so there is no table reload churn against
   the softmax exps.
 - Causal masking of the diagonal block = DVE multiply with a precomputed
   0/1 bf16 mask (built once with affine_select at init).
 - Softmax 1/denominator via reciprocal_approx_fast (~18 bits, plenty under
   bf16 downstream noise).
"""

import numpy as np
import ml_dtypes

B, S, D = 4, 2048, 1024
H, KVH, HD = 16, 4, 64
SC = S // 128   # 16 sequence chunks
DC = D // 128   # 8 d_model chunks
QD = 512        # local q dims (8 heads)
EPS = float(np.finfo(np.float32).eps)
ROPE_BASE = 10000.0

_NC_CACHE = {}
_LAST = None  # BassKernelResults of the last run (for test harness introspection)


def _build_bass():
    import concourse.bacc as bacc
    import concourse.mybir as mybir
    import concourse.tile as tile
    from concourse.masks import make_identity

    dt = mybir.dt
    f32, bf16 = dt.float32, dt.bfloat16
    Alu = mybir.AluOpType
    Act = mybir.ActivationFunctionType
    Ax = mybir.AxisListType

    nc = bacc.Bacc("TRN2", target_bir_lowering=False)

    xTd = nc.dram_tensor("xT", [D, S], bf16, kind="ExternalInput")
    wqd = nc.dram_tensor("wq", [D, QD], bf16, kind="ExternalInput")
    wkvd = nc.dram_tensor("wkv", [D, 256], bf16, kind="ExternalInput")
    wpd = nc.dram_tensor("wp", [QD, D], bf16, kind="ExternalInput")
    cqd = nc.dram_tensor("cq", [S, QD], bf16, kind="ExternalInput")
    sqd = nc.dram_tensor("sq", [S, QD], bf16, kind="ExternalInput")
    ckd = nc.dram_tensor("ck", [S, 128], bf16, kind="ExternalInput")
    skd = nc.dram_tensor("sk", [S, 128], bf16, kind="ExternalInput")
    yd = nc.dram_tensor("y", [S, D], f32, kind="ExternalOutput")

    with tile.TileContext(nc) as tc:
        with (
            tc.tile_pool(name="per", bufs=1) as per,
            tc.tile_pool(name="wk", bufs=3) as wk,
            tc.tile_pool(name="ep", bufs=4) as ep,
            tc.tile_pool(name="pmm", bufs=3, space="PSUM") as pmm,
            tc.tile_pool(name="po", bufs=2, space="PSUM") as po,
        ):
            xt = [per.tile([128, S], bf16, tag=f"xt{k}", name=f"xt{k}") for k in range(DC)]
            wq = [per.tile([128, QD], bf16, tag=f"wq{k}", name=f"wq{k}") for k in range(DC)]
            wkv = [per.tile([128, 256], bf16, tag=f"wkv{k}", name=f"wkv{k}") for k in range(DC)]
            wp = per.tile([128, 4 * D], bf16, tag="wp")
            cq = [per.tile([128, QD], bf16, tag=f"cq{i}", name=f"cq{i}") for i in range(SC)]
            sq = [per.tile([128, QD], bf16, tag=f"sq{i}", name=f"sq{i}") for i in range(SC)]
            ck = [per.tile([128, 128], bf16, tag=f"ck{i}", name=f"ck{i}") for i in range(SC)]
            sk = [per.tile([128, 128], bf16, tag=f"sk{i}", name=f"sk{i}") for i in range(SC)]
            ident = per.tile([128, 128], bf16, tag="ident")
            maskc = per.tile([128, 1024], bf16, tag="maskc")
            ones1 = per.tile([1, 64], bf16, tag="ones1")
            half_t = per.tile([128, 1], f32, tag="half_t")
            QT = per.tile([128, 4 * S], bf16, tag="QT")
            KT = per.tile([128, S], bf16, tag="KT")
            VV = per.tile([128, SC * 130], bf16, tag="VV")
            YT = per.tile([128, 4 * S], bf16, tag="YT")

            # --- input DMAs, critical-path-first issue order ---
            for k in range(DC):
                nc.sync.dma_start(xt[k][:], xTd[k * 128:(k + 1) * 128, :])
                nc.sync.dma_start(wq[k][:], wqd[k * 128:(k + 1) * 128, :])
                nc.sync.dma_start(wkv[k][:], wkvd[k * 128:(k + 1) * 128, :])
            for i in range(SC):
                nc.scalar.dma_start(cq[i][:], cqd[i * 128:(i + 1) * 128, :])
                nc.scalar.dma_start(sq[i][:], sqd[i * 128:(i + 1) * 128, :])
                nc.scalar.dma_start(ck[i][:], ckd[i * 128:(i + 1) * 128, :])
                nc.scalar.dma_start(sk[i][:], skd[i * 128:(i + 1) * 128, :])
            for m in range(4):
                nc.scalar.dma_start(wp[:, m * D:(m + 1) * D], wpd[m * 128:(m + 1) * 128, :])

            make_identity(nc, ident[:])
            nc.vector.memset(ones1[:], 1.0)
            nc.vector.memset(half_t[:], 0.5)
            # ones columns of VV (softmax denominator rows), set once
            vv3 = VV[:].rearrange("p (i c) -> p i c", c=130)
            nc.vector.memset(vv3[:, :, 64:65], 1.0)
            nc.vector.memset(vv3[:, :, 129:130], 1.0)
            # causal 0/1 mask for the diagonal block: keep where q >= k
            nc.vector.memset(maskc[:], 1.0)
            m3 = maskc[:].rearrange("p (b q) -> p b q", q=128)
            nc.gpsimd.affine_select(
                m3, m3, pattern=[[0, 8], [1, 128]],
                compare_op=Alu.is_ge, fill=0.0, base=0,
                channel_multiplier=-1)

            # --- prep stages -------------------------------------------------
            state = {}

            def prepA_q(i):
                qkv_ps = pmm.tile([128, 1024], f32, tag="mm")
                for k in range(DC):
                    nc.tensor.matmul(
                        qkv_ps[:, 0:QD],
                        xt[k][:, i * 128:(i + 1) * 128],
                        wq[k][:],
                        start=(k == 0), stop=(k == DC - 1),
                    )
                state[i] = [qkv_ps]

            def prepA_kv(i):
                qkv_ps = state[i][0]
                for k in range(DC):
                    nc.tensor.matmul(
                        qkv_ps[:, QD:QD + 256],
                        xt[k][:, i * 128:(i + 1) * 128],
                        wkv[k][:],
                        start=(k == 0), stop=(k == DC - 1),
                    )

            def prepA_post(i):
                qkv_ps = state[i][0]
                q_ps = qkv_ps[:, 0:QD]
                k_ps = qkv_ps[:, QD:QD + 128]
                # squares on DVE (keeps ACT free for softmax exps); DVE can
                # only read one PSUM operand per op, so stage a copy in SBUF
                qc = wk.tile([128, QD], f32, tag="qc")
                nc.vector.tensor_copy(qc[:], q_ps)
                kc = wk.tile([128, 128], f32, tag="kc")
                nc.vector.tensor_copy(kc[:], k_ps)
                q2 = wk.tile([128, QD], f32, tag="q2")
                nc.vector.tensor_tensor(q2[:], q_ps, qc[:], Alu.mult)
                k2 = wk.tile([128, 128], f32, tag="k2")
                nc.vector.tensor_tensor(k2[:], k_ps, kc[:], Alu.mult)
                ss = wk.tile([128, 10], f32, tag="ss")
                nc.vector.tensor_reduce(
                    ss[:, 0:8], q2[:].rearrange("p (h f) -> p h f", h=8), Ax.X, Alu.add)
                nc.vector.tensor_reduce(
                    ss[:, 8:10], k2[:].rearrange("p (h f) -> p h f", h=2), Ax.X, Alu.add)
                lnv = wk.tile([128, 10], f32, tag="lnv")
                nc.vector.tensor_scalar(lnv[:], ss[:], 1.0 / HD, EPS, Alu.mult, Alu.add)
                # rsqrt(v): ACT-exp seed exp(0.5-0.5v) ~ v^-0.5 near 1 (keeps the
                # exp table set resident), then 2 Newton steps on DVE
                rs = wk.tile([128, 10], f32, tag="rs")
                nc.scalar.activation(rs[:], lnv[:], Act.Exp, scale=-0.5, bias=half_t[:])
                nt = wk.tile([128, 20], f32, tag="nt")
                for it in range(2):
                    t0 = nt[:, it * 10: it * 10 + 10]
                    nc.vector.tensor_tensor(t0, rs[:], rs[:], Alu.mult)
                    nc.vector.tensor_tensor(t0, t0, lnv[:], Alu.mult)
                    nc.vector.tensor_scalar(t0, t0, -0.5, 1.5, Alu.mult, Alu.add)
                    nc.vector.tensor_tensor(rs[:], rs[:], t0, Alu.mult)
                qn = wk.tile([128, QD], bf16, tag="qn", bufs=4)
                nc.vector.tensor_tensor(
                    qn[:].rearrange("p (h f) -> p h f", h=8),
                    q_ps.rearrange("p (h f) -> p h f", h=8),
                    rs[:, 0:8].rearrange("p (h o) -> p h o", o=1).to_broadcast((128, 8, 64)),
                    Alu.mult)
                kn = wk.tile([128, 128], bf16, tag="kn", bufs=4)
                nc.vector.tensor_tensor(
                    kn[:].rearrange("p (h f) -> p h f", h=2),
                    k_ps.rearrange("p (h f) -> p h f", h=2),
                    rs[:, 8:10].rearrange("p (h o) -> p h o", o=1).to_broadcast((128, 2, 64)),
                    Alu.mult)
                vt = VV[:, i * 130:(i + 1) * 130]
                nc.vector.tensor_copy(vt[:, 0:64], qkv_ps[:, QD + 128:QD + 192])
                nc.vector.tensor_copy(vt[:, 65:129], qkv_ps[:, QD + 192:QD + 256])
                state[i] += [qn, kn]

            def prepB(i):
                _, qn, kn = state[i]
                # q rope on gpsimd
                r1 = wk.tile([128, QD], bf16, tag="r1")
                nc.gpsimd.tensor_tensor(r1[:], qn[:], cq[i][:], Alu.mult)
                r2 = wk.tile([128, QD], bf16, tag="r2")
                qn3 = qn[:].rearrange("p (h t f) -> p h t f", t=2, f=32)
                sq3 = sq[i][:].rearrange("p (h t f) -> p h t f", t=2, f=32)
                r23 = r2[:].rearrange("p (h t f) -> p h t f", t=2, f=32)
                nc.gpsimd.tensor_tensor(r23[:, :, 0, :], qn3[:, :, 1, :], sq3[:, :, 0, :], Alu.mult)
                nc.gpsimd.tensor_tensor(r23[:, :, 1, :], qn3[:, :, 0, :], sq3[:, :, 1, :], Alu.mult)
                qr = wk.tile([128, QD], bf16, tag="qr", bufs=4)
                nc.gpsimd.tensor_tensor(qr[:], r1[:], r2[:], Alu.add)
                # k rope on DVE (gpsimd is the busier engine)
                rk1 = wk.tile([128, 128], bf16, tag="rk1")
                nc.vector.tensor_tensor(rk1[:], kn[:], ck[i][:], Alu.mult)
                rk2 = wk.tile([128, 128], bf16, tag="rk2")
                kn3 = kn[:].rearrange("p (h t f) -> p h t f", t=2, f=32)
                sk3 = sk[i][:].rearrange("p (h t f) -> p h t f", t=2, f=32)
                rk23 = rk2[:].rearrange("p (h t f) -> p h t f", t=2, f=32)
                nc.vector.tensor_tensor(rk23[:, :, 0, :], kn3[:, :, 1, :], sk3[:, :, 0, :], Alu.mult)
                nc.vector.tensor_tensor(rk23[:, :, 1, :], kn3[:, :, 0, :], sk3[:, :, 1, :], Alu.mult)
                kr = wk.tile([128, 128], bf16, tag="kr", bufs=4)
                nc.vector.tensor_tensor(kr[:], rk1[:], rk2[:], Alu.add)
                state[i] += [qr, kr]

            def prepC(i):
                _, _, _, qr, kr = state.pop(i)
                t_ps = pmm.tile([128, 640], bf16, tag="mm")
                for m in range(4):
                    nc.tensor.transpose(t_ps[:, m * 128:(m + 1) * 128],
                                        qr[:, m * 128:(m + 1) * 128], ident[:])
                nc.tensor.transpose(t_ps[:, 512:640], kr[:], ident[:])
                nc.vector.tensor_copy(
                    QT[:].rearrange("p (m s) -> p m s", m=4)[:, :, i * 128:(i + 1) * 128],
                    t_ps[:, 0:512].rearrange("p (m s) -> p m s", m=4))
                nc.vector.tensor_copy(KT[:, i * 128:(i + 1) * 128], t_ps[:, 512:640])

            # --- attention row ----------------------------------------------
            def score_step(i, j, oa, ob, qt0, qt1):
                s_ps = pmm.tile([128, 1024], f32, tag="mm")
                nc.tensor.matmul(s_ps[:, 0:512], KT[0:64, j * 128:(j + 1) * 128], qt0,
                                 start=True, stop=True)
                nc.tensor.matmul(s_ps[:, 512:1024], KT[64:128, j * 128:(j + 1) * 128], qt1,
                                 start=True, stop=True)
                et = ep.tile([128, 1024], bf16, tag="e")
                nc.scalar.activation(et[:], s_ps[:], Act.Exp)
                if j == i:
                    # zero strictly-above-diagonal weights (k > q) in-block
                    nc.vector.tensor_tensor(et[:], et[:], maskc[:], Alu.mult)
                nc.tensor.matmul(oa[:], VV[:, j * 130: j * 130 + 65], et[:, 0:512],
                                 start=(j == 0), stop=(j == i))
                nc.tensor.matmul(ob[:], VV[:, j * 130 + 65: j * 130 + 130], et[:, 512:1024],
                                 start=(j == 0), stop=(j == i))

            def normalize(i, oa, ob):
                rcs = []
                for o_ps in (oa, ob):
                    # the bit-trick custom op misreads PSUM; stage denom in SBUF
                    dcp = wk.tile([1, QD], f32, tag="dcp")
                    nc.vector.tensor_copy(dcp[:], o_ps[64:65, :])
                    rc = wk.tile([1, QD], f32, tag="rc")
                    nc.vector.reciprocal_approx_fast(rc[:], dcp[:])
                    rb = wk.tile([64, QD], f32, tag="rb")
                    nc.gpsimd.partition_broadcast(rb[:], rc[:], channels=64)
                    rcs.append(rb)
                for g, o_ps in ((0, oa), (1, ob)):
                    out_ap = YT[g * 64:(g + 1) * 64, :].rearrange(
                        "p (m s) -> p m s", m=4)[:, :, i * 128:(i + 1) * 128]
                    nc.vector.tensor_tensor(
                        out_ap,
                        o_ps[0:64, :].rearrange("p (m q) -> p m q", m=4),
                        rcs[g][:].rearrange("p (m q) -> p m q", m=4),
                        Alu.mult)

            def proj(ip):
                op_ps = pmm.tile([128, 1024], f32, tag="mm")
                for dh in range(2):
                    for m in range(4):
                        nc.tensor.matmul(
                            op_ps[:, dh * 512:(dh + 1) * 512],
                            YT[:, m * S + ip * 128: m * S + (ip + 1) * 128],
                            wp[:, m * D + dh * 512: m * D + (dh + 1) * 512],
                            start=(m == 0), stop=(m == 3))
                osb = wk.tile([128, 1024], f32, tag="osb")
                nc.vector.tensor_copy(osb[:], op_ps[:])
                nc.sync.dma_start(yd[ip * 128:(ip + 1) * 128, :], osb[:])

            # --- fused pipeline ---------------------------------------------
            prepA_q(0); prepA_kv(0); prepA_post(0); prepB(0); prepC(0)
            for i in range(SC):
                oa = po.tile([65, QD], f32, tag="o")
                ob = po.tile([65, QD], f32, tag="o")
                qt0 = QT[0:64, :].rearrange("p (m s) -> p m s", m=4)[:, :, i * 128:(i + 1) * 128]
                qt1 = QT[64:128, :].rearrange("p (m s) -> p m s", m=4)[:, :, i * 128:(i + 1) * 128]
                # interleave prep of chunk i+1 between the first score steps so
                # the exp stream starts immediately and PE stays fed
                todo = []
                if i + 1 < SC:
                    todo = [lambda: prepA_q(i + 1), lambda: prepA_kv(i + 1),
                            lambda: prepA_post(i + 1), lambda: prepB(i + 1),
                            lambda: prepC(i + 1)]
                for j in range(i + 1):
                    score_step(i, j, oa, ob, qt0, qt1)
                    if todo:
                        todo.pop(0)()
                for t in todo:
                    t()
                normalize(i, oa, ob)
                if i > 0:
                    proj(i - 1)
            proj(SC - 1)

    nc.compile()
    return nc


def _get_nc():
    if "nc" not in _NC_CACHE:
        _NC_CACHE["nc"] = _build_bass()
    return _NC_CACHE["nc"]


def _core_inputs(xb, Wq, Wk, Wv, Wproj, q_gain, g):
    bf = ml_dtypes.bfloat16
    qorder = [8 * g + o for o in (0, 4, 1, 5, 2, 6, 3, 7)]

    xT = np.ascontiguousarray(np.asarray(xb, np.float32).T).astype(bf)
    Wq_l = np.concatenate([Wq[h * 64:(h + 1) * 64] for h in qorder], 0)  # [512, D]
    wq = np.ascontiguousarray(Wq_l.T).astype(bf)
    Wk_l = Wk[2 * g * 64:(2 * g + 2) * 64]  # [128, D]
    Wv_l = Wv[2 * g * 64:(2 * g + 2) * 64]
    wkv = np.ascontiguousarray(np.concatenate([Wk_l, Wv_l], 0).T).astype(bf)
    cols = np.array([(8 * g + m + 4 * half) * 64 + f
                     for m in range(4) for half in range(2) for f in range(64)])
    wp = np.ascontiguousarray(Wproj[:, cols].T).astype(bf)  # [512, D]

    inv = (1.0 / (ROPE_BASE ** (np.arange(0, HD, 2, dtype=np.float32) / HD))).astype(np.float32)
    th = np.arange(S, dtype=np.float32)[:, None] * inv[None, :]
    cos, sin = np.cos(th).astype(np.float32), np.sin(th).astype(np.float32)
    cfull = np.concatenate([cos, cos], 1)       # [S, 64]
    sfull = np.concatenate([sin, -sin], 1)      # [S, 64] (signs baked)
    scale_q = np.asarray(q_gain, np.float32)[qorder] / np.float32(np.sqrt(HD))
    cq = np.concatenate([cfull * sc for sc in scale_q], 1).astype(bf)
    sq = np.concatenate([sfull * sc for sc in scale_q], 1).astype(bf)
    ck = np.concatenate([cfull, cfull], 1).astype(bf)
    sk = np.concatenate([sfull, sfull], 1).astype(bf)

    return {"xT": xT, "wq": wq, "wkv": wkv, "wp": wp,
            "cq": cq, "sq": sq, "ck": ck, "sk": sk}


def kernel(x, Wq, Wk, Wv, Wproj, q_gain):
    global _LAST
    x = np.asarray(x, np.float32)
    Wq = np.asarray(Wq, np.float32)
    Wk = np.asarray(Wk, np.float32)
    Wv = np.asarray(Wv, np.float32)
    Wproj = np.asarray(Wproj, np.float32)
    q_gain = np.asarray(q_gain, np.float32)

    nc = _get_nc()
    in_maps = []
    for c in range(8):
        b, g = divmod(c, 2)
        in_maps.append(_core_inputs(x[b], Wq, Wk, Wv, Wproj, q_gain, g))

    from concourse.bass_utils import run_bass_kernel_spmd
    res = run_bass_kernel_spmd(nc, in_maps, core_ids=list(range(8)))
    _LAST = res

    y = np.empty((B, S, D), np.float32)
    for b in range(B):
        y[b] = res.results[2 * b]["y"] + res.results[2 * b + 1]["y"]
    return y


# revision 23
# speedup vs baseline: 1.1964x; 1.1964x over previous
"""Causal GQA self-attention (RMS-normed QK + RoPE + softmax + proj) on 8 trn2 cores.

Sharding: core c = (batch b = c//2, head-group g = c%2).  Each core computes
batch b, q-heads {8g..8g+7}, kv-heads {2g, 2g+1}, and a partial output
projection using Wproj columns for those heads; the host sums the two
partials per batch.

Device layout notes (per core):
 - All matmul operands bf16; accumulation fp32 in PSUM.
 - x is pre-transposed on host: xT [D=1024, S=2048].
 - Q and K are projected together into one [s, 640] strip (512 q dims +
   128 k dims) so RMS-norm scaling and RoPE each take ONE instruction over
   the whole strip (10 heads x 64), then 5 PE-transposes produce the
   [dim, s] layout (4 q blocks + 1 k block) written with one strided copy.
 - Local q-dim order pairs head m of group0 with head m of group1:
   [h0,h4,h1,h5,h2,h6,h3,h7] so scores for the two kv-heads can be computed
   with two row-tiled (K=64) matmuls sharing the PE array.
 - Scores are computed TRANSPOSED: S^T [k_s, (pair m, q)] so exp output
   feeds PV directly as the moving operand; V is augmented with a ones
   column so row 64 of the PV accumulator is the softmax denominator.
 - Single fused loop: prep of chunk i+1 and the deferred projection of
   chunk i-1 are interleaved between the score steps of attention row i,
   sharing one rotating PSUM pool, so the in-order PE queue never stalls
   behind a dependency (keeps the HAM clock gate at 2.4 GHz).
 - rsqrt(v) for RMS norm = one gpsimd pow(v, -0.5) (vpowf, fp32-accurate).
 - Causal masking of the diagonal block = DVE multiply with a precomputed
   0/1 bf16 mask (built once with affine_select at init).
 - Softmax normalize: copy the [65, 512] PV accumulators into one SBUF
   staging tile immediately (frees the PSUM accumulators for the next row),
   one reciprocal_approx_fast over both groups' denominators, one gpsimd
   partition_broadcast, two multiplies into Y^T.
 - Output projection PSUM is drained by the Scalar engine (ACT copy) to
   bf16 and DMA'd out as bf16 partials (host sums in fp32).
"""

import numpy as np
import ml_dtypes

B, S, D = 4, 2048, 1024
H, KVH, HD = 16, 4, 64
SC = S // 128   # 16 sequence chunks
DC = D // 128   # 8 d_model chunks
QD = 512        # local q dims (8 heads)
QK = 640        # q dims + k dims in the fused norm/rope strip
EPS = float(np.finfo(np.float32).eps)
ROPE_BASE = 10000.0

_NC_CACHE = {}
_LAST = None  # BassKernelResults of the last run (for test harness introspection)


def _build_bass():
    import concourse.bacc as bacc
    import concourse.mybir as mybir
    import concourse.tile as tile
    from concourse.masks import make_identity

    dt = mybir.dt
    f32, bf16 = dt.float32, dt.bfloat16
    Alu = mybir.AluOpType
    Act = mybir.ActivationFunctionType
    Ax = mybir.AxisListType

    nc = bacc.Bacc("TRN2", target_bir_lowering=False)

    xTd = nc.dram_tensor("xT", [D, S], bf16, kind="ExternalInput")
    wqd = nc.dram_tensor("wq", [D, QD], bf16, kind="ExternalInput")
    wkvd = nc.dram_tensor("wkv", [D, 256], bf16, kind="ExternalInput")
    wpd = nc.dram_tensor("wp", [QD, D], bf16, kind="ExternalInput")
    ctd = nc.dram_tensor("ct", [S, QK], bf16, kind="ExternalInput")
    std = nc.dram_tensor("st", [S, QK], bf16, kind="ExternalInput")
    yd = nc.dram_tensor("y", [S, D], bf16, kind="ExternalOutput")

    with tile.TileContext(nc) as tc:
        with (
            tc.tile_pool(name="per", bufs=1) as per,
            tc.tile_pool(name="wk", bufs=3) as wk,
            tc.tile_pool(name="ep", bufs=4) as ep,
            tc.tile_pool(name="pmm", bufs=3, space="PSUM") as pmm,
            tc.tile_pool(name="po", bufs=2, space="PSUM") as po,
        ):
            xt = [per.tile([128, S], bf16, tag=f"xt{k}", name=f"xt{k}") for k in range(DC)]
            wq = [per.tile([128, QD], bf16, tag=f"wq{k}", name=f"wq{k}") for k in range(DC)]
            wkv = [per.tile([128, 256], bf16, tag=f"wkv{k}", name=f"wkv{k}") for k in range(DC)]
            wp = per.tile([128, 4 * D], bf16, tag="wp")
            ct = [per.tile([128, QK], bf16, tag=f"ct{i}", name=f"ct{i}") for i in range(SC)]
            st = [per.tile([128, QK], bf16, tag=f"st{i}", name=f"st{i}") for i in range(SC)]
            ident = per.tile([128, 128], bf16, tag="ident")
            maskc = per.tile([128, 1024], bf16, tag="maskc")
            mhalf = per.tile([128, 10], f32, tag="mhalf")
            ones1 = per.tile([1, 64], bf16, tag="ones1")
            QKT = per.tile([128, 5 * S], bf16, tag="QKT")
            VV = per.tile([128, SC * 130], bf16, tag="VV")
            YT = per.tile([128, 4 * S], bf16, tag="YT")
            KTOFF = 4 * S

            def tab_dma(i):
                nc.sync.dma_start(ct[i][:], ctd[i * 128:(i + 1) * 128, :])
                nc.sync.dma_start(st[i][:], std[i * 128:(i + 1) * 128, :])

            # --- input DMAs, critical-path-first issue order ---
            for k in range(DC):
                nc.sync.dma_start(xt[k][:], xTd[k * 128:(k + 1) * 128, :])
                nc.sync.dma_start(wq[k][:], wqd[k * 128:(k + 1) * 128, :])
                nc.sync.dma_start(wkv[k][:], wkvd[k * 128:(k + 1) * 128, :])
            tab_dma(0)
            tab_dma(1)
            for m in range(4):
                nc.sync.dma_start(wp[:, m * D:(m + 1) * D], wpd[m * 128:(m + 1) * 128, :])

            make_identity(nc, ident[:])
            nc.vector.memset(mhalf[:], -0.5)
            nc.vector.memset(ones1[:], 1.0)
            # ones columns of VV (softmax denominator rows), set once
            vv3 = VV[:].rearrange("p (i c) -> p i c", c=130)
            nc.vector.memset(vv3[:, :, 64:65], 1.0)
            nc.vector.memset(vv3[:, :, 129:130], 1.0)
            # causal 0/1 mask for the diagonal block: keep where q >= k
            nc.vector.memset(maskc[:], 1.0)
            m3 = maskc[:].rearrange("p (b q) -> p b q", q=128)
            nc.gpsimd.affine_select(
                m3, m3, pattern=[[0, 8], [1, 128]],
                compare_op=Alu.is_ge, fill=0.0, base=0,
                channel_multiplier=-1)

            # --- prep stages -------------------------------------------------
            state = {}

            def prepA_q(i):
                qkv_ps = pmm.tile([128, 1024], f32, tag="mm")
                for k in range(DC):
                    nc.tensor.matmul(
                        qkv_ps[:, 0:QD],
                        xt[k][:, i * 128:(i + 1) * 128],
                        wq[k][:],
                        start=(k == 0), stop=(k == DC - 1),
                    )
                state[i] = [qkv_ps]

            def prepA_kv(i):
                qkv_ps = state[i][0]
                for k in range(DC):
                    nc.tensor.matmul(
                        qkv_ps[:, QD:QD + 256],
                        xt[k][:, i * 128:(i + 1) * 128],
                        wkv[k][:],
                        start=(k == 0), stop=(k == DC - 1),
                    )

            def prepA_post(i):
                qkv_ps = state[i][0]
                # sum of squares per head (10 heads x 64 dims) for RMS norm;
                # square runs on ACT (idle-ish), reduce + scale on DVE,
                # rsqrt = one gpsimd pow(v, -0.5)
                sq = wk.tile([128, QK], f32, tag="sq", bufs=2)
                nc.scalar.activation(sq[:], qkv_ps[:, 0:QK], Act.Square)
                ss = wk.tile([128, 10], f32, tag="ss")
                nc.vector.tensor_reduce(
                    ss[:], sq[:].rearrange("p (h f) -> p h f", h=10), Ax.X, Alu.add)
                lnv = wk.tile([128, 10], f32, tag="lnv")
                nc.vector.tensor_scalar(lnv[:], ss[:], 1.0 / HD, EPS, Alu.mult, Alu.add)
                rs = wk.tile([128, 10], f32, tag="rs")
                nc.gpsimd.tensor_tensor(rs[:], lnv[:], mhalf[:], Alu.pow)
                # normalized q||k strip in ONE broadcast multiply
                qkn = wk.tile([128, QK], bf16, tag="qkn", bufs=4)
                nc.vector.tensor_tensor(
                    qkn[:].rearrange("p (h f) -> p h f", h=10),
                    qkv_ps[:, 0:QK].rearrange("p (h f) -> p h f", h=10),
                    rs[:].rearrange("p (h o) -> p h o", o=1).to_broadcast((128, 10, 64)),
                    Alu.mult)
                # V (2 heads) into VV columns, one strided copy
                vt = VV[:, i * 130:(i + 1) * 130].rearrange("p (g c) -> p g c", g=2)[:, :, 0:64]
                nc.vector.tensor_copy(
                    vt, qkv_ps[:, QK:QK + 128].rearrange("p (g c) -> p g c", g=2))
                state[i] += [qkn]

            def prepB(i):
                _, qkn = state[i]
                # RoPE over the whole q||k strip: 4 ops total
                r1 = wk.tile([128, QK], bf16, tag="r1", bufs=2)
                nc.gpsimd.tensor_tensor(r1[:], qkn[:], ct[i][:], Alu.mult)
                r2 = wk.tile([128, QK], bf16, tag="r2", bufs=2)
                qk3 = qkn[:].rearrange("p (h t f) -> p h t f", t=2, f=32)
                st3 = st[i][:].rearrange("p (h t f) -> p h t f", t=2, f=32)
                r23 = r2[:].rearrange("p (h t f) -> p h t f", t=2, f=32)
                nc.vector.tensor_tensor(r23[:, :, 0, :], qk3[:, :, 1, :], st3[:, :, 0, :], Alu.mult)
                nc.vector.tensor_tensor(r23[:, :, 1, :], qk3[:, :, 0, :], st3[:, :, 1, :], Alu.mult)
                qkr = wk.tile([128, QK], bf16, tag="qkr", bufs=4)
                nc.gpsimd.tensor_tensor(qkr[:], r1[:], r2[:], Alu.add)
                state[i] += [qkr]

            def prepC(i):
                _, _, qkr = state.pop(i)
                t_ps = pmm.tile([128, QK], bf16, tag="mm")
                for m in range(5):
                    nc.tensor.transpose(t_ps[:, m * 128:(m + 1) * 128],
                                        qkr[:, m * 128:(m + 1) * 128], ident[:])
                nc.vector.tensor_copy(
                    QKT[:].rearrange("p (m s) -> p m s", m=5)[:, :, i * 128:(i + 1) * 128],
                    t_ps[:].rearrange("p (m s) -> p m s", m=5))

            # --- attention row ----------------------------------------------
            def score_step(i, j, oa, ob, qt0, qt1):
                s_ps = pmm.tile([128, 1024], f32, tag="mm")
                nc.tensor.matmul(s_ps[:, 0:512],
                                 QKT[0:64, KTOFF + j * 128: KTOFF + (j + 1) * 128], qt0,
                                 start=True, stop=True)
                nc.tensor.matmul(s_ps[:, 512:1024],
                                 QKT[64:128, KTOFF + j * 128: KTOFF + (j + 1) * 128], qt1,
                                 start=True, stop=True)
                et = ep.tile([128, 1024], bf16, tag="e")
                nc.scalar.activation(et[:], s_ps[:], Act.Exp)
                if j == i:
                    # zero strictly-above-diagonal weights (k > q) in-block
                    nc.vector.tensor_tensor(et[:], et[:], maskc[:], Alu.mult)
                nc.tensor.matmul(oa[:], VV[:, j * 130: j * 130 + 65], et[:, 0:512],
                                 start=(j == 0), stop=(j == i))
                nc.tensor.matmul(ob[:], VV[:, j * 130 + 65: j * 130 + 130], et[:, 512:1024],
                                 start=(j == 0), stop=(j == i))

            def normalize(i, oa, ob):
                # stage both accumulators in SBUF right away (frees the PSUM
                # 'o' slots for the next row), then one reciprocal over both
                # groups' denominators and one partition broadcast
                oc = wk.tile([65, 1024], f32, tag="oc", bufs=2)
                nc.vector.tensor_copy(oc[:, 0:512], oa[:])
                nc.vector.tensor_copy(oc[:, 512:1024], ob[:])
                # the custom-DVE reciprocal mishandles inputs at a nonzero
                # partition offset: stage the denominator row at partition 0
                dcp = wk.tile([1, 1024], f32, tag="dcp", bufs=2)
                nc.vector.tensor_copy(dcp[:], oc[64:65, :])
                rc = wk.tile([1, 1024], f32, tag="rc", bufs=2)
                nc.vector.reciprocal_approx_fast(rc[:, 0:512], dcp[:, 0:512])
                nc.vector.reciprocal_approx_fast(rc[:, 512:1024], dcp[:, 512:1024])
                rb = wk.tile([64, 1024], f32, tag="rb", bufs=2)
                nc.gpsimd.partition_broadcast(rb[:, 0:512], rc[:, 0:512], channels=64)
                nc.gpsimd.partition_broadcast(rb[:, 512:1024], rc[:, 512:1024], channels=64)
                for g in range(2):
                    out_ap = YT[g * 64:(g + 1) * 64, :].rearrange(
                        "p (m s) -> p m s", m=4)[:, :, i * 128:(i + 1) * 128]
                    nc.vector.tensor_tensor(
                        out_ap,
                        oc[0:64, g * 512:(g + 1) * 512].rearrange("p (m q) -> p m q", m=4),
                        rb[:, g * 512:(g + 1) * 512].rearrange("p (m q) -> p m q", m=4),
                        Alu.mult)

            def proj(ip):
                op_ps = pmm.tile([128, 1024], f32, tag="mm")
                for dh in range(2):
                    for m in range(4):
                        nc.tensor.matmul(
                            op_ps[:, dh * 512:(dh + 1) * 512],
                            YT[:, m * S + ip * 128: m * S + (ip + 1) * 128],
                            wp[:, m * D + dh * 512: m * D + (dh + 1) * 512],
                            start=(m == 0), stop=(m == 3))
                osb = wk.tile([128, 1024], bf16, tag="osb", bufs=2)
                nc.scalar.copy(osb[:], op_ps[:])
                nc.sync.dma_start(yd[ip * 128:(ip + 1) * 128, :], osb[:])

            # --- fused pipeline ---------------------------------------------
            prepA_q(0); prepA_kv(0); prepA_post(0); prepB(0); prepC(0)
            for i in range(SC):
                oa = po.tile([65, QD], f32, tag="o")
                ob = po.tile([65, QD], f32, tag="o")
                qk5 = QKT[:].rearrange("p (m s) -> p m s", m=5)
                qt0 = qk5[0:64, 0:4, i * 128:(i + 1) * 128]
                qt1 = qk5[64:128, 0:4, i * 128:(i + 1) * 128]
                # interleave prep of chunk i+1 and proj of chunk i-1 between
                # the score steps so every engine queue stays fed in order
                todo = []
                if i + 1 < SC:
                    todo += [lambda: prepA_q(i + 1), lambda: prepA_kv(i + 1)]
                if i > 0:
                    todo += [lambda: proj(i - 1)]
                if i + 1 < SC:
                    todo += [lambda: prepA_post(i + 1), lambda: prepB(i + 1),
                             lambda: prepC(i + 1)]
                if i + 2 < SC:
                    todo += [lambda: tab_dma(i + 2)]
                for j in range(i + 1):
                    score_step(i, j, oa, ob, qt0, qt1)
                    if todo:
                        todo.pop(0)()
                normalize(i, oa, ob)
                # leftovers (short early rows) go after normalize so the
                # row-tail DVE chain stays contiguous
                for t in todo:
                    t()
            proj(SC - 1)

    nc.compile()
    return nc


def _get_nc():
    if "nc" not in _NC_CACHE:
        _NC_CACHE["nc"] = _build_bass()
    return _NC_CACHE["nc"]


def _core_inputs(xb, Wq, Wk, Wv, Wproj, q_gain, g):
    bf = ml_dtypes.bfloat16
    qorder = [8 * g + o for o in (0, 4, 1, 5, 2, 6, 3, 7)]

    xT = np.ascontiguousarray(np.asarray(xb, np.float32).T).astype(bf)
    Wq_l = np.concatenate([Wq[h * 64:(h + 1) * 64] for h in qorder], 0)  # [512, D]
    wq = np.ascontiguousarray(Wq_l.T).astype(bf)
    Wk_l = Wk[2 * g * 64:(2 * g + 2) * 64]  # [128, D]
    Wv_l = Wv[2 * g * 64:(2 * g + 2) * 64]
    wkv = np.ascontiguousarray(np.concatenate([Wk_l, Wv_l], 0).T).astype(bf)
    cols = np.array([(8 * g + m + 4 * half) * 64 + f
                     for m in range(4) for half in range(2) for f in range(64)])
    wp = np.ascontiguousarray(Wproj[:, cols].T).astype(bf)  # [512, D]

    inv = (1.0 / (ROPE_BASE ** (np.arange(0, HD, 2, dtype=np.float32) / HD))).astype(np.float32)
    th = np.arange(S, dtype=np.float32)[:, None] * inv[None, :]
    cos, sin = np.cos(th).astype(np.float32), np.sin(th).astype(np.float32)
    cfull = np.concatenate([cos, cos], 1)       # [S, 64]
    sfull = np.concatenate([sin, -sin], 1)      # [S, 64] (signs baked)
    scale_q = np.asarray(q_gain, np.float32)[qorder] / np.float32(np.sqrt(HD))
    # merged q||k rope tables [S, 640]: 8 q heads (scale baked) + 2 k heads
    ct = np.concatenate([cfull * sc for sc in scale_q] + [cfull, cfull], 1).astype(bf)
    st = np.concatenate([sfull * sc for sc in scale_q] + [sfull, sfull], 1).astype(bf)

    return {"xT": xT, "wq": wq, "wkv": wkv, "wp": wp, "ct": ct, "st": st}


def kernel(x, Wq, Wk, Wv, Wproj, q_gain):
    global _LAST
    x = np.asarray(x, np.float32)
    Wq = np.asarray(Wq, np.float32)
    Wk = np.asarray(Wk, np.float32)
    Wv = np.asarray(Wv, np.float32)
    Wproj = np.asarray(Wproj, np.float32)
    q_gain = np.asarray(q_gain, np.float32)

    nc = _get_nc()
    in_maps = []
    for c in range(8):
        b, g = divmod(c, 2)
        in_maps.append(_core_inputs(x[b], Wq, Wk, Wv, Wproj, q_gain, g))

    from concourse.bass_utils import run_bass_kernel_spmd
    res = run_bass_kernel_spmd(nc, in_maps, core_ids=list(range(8)))
    _LAST = res

    y = np.empty((B, S, D), np.float32)
    for b in range(B):
        y[b] = (np.asarray(res.results[2 * b]["y"], np.float32)
                + np.asarray(res.results[2 * b + 1]["y"], np.float32))
    return y
